# revision 16
# baseline (speedup 1.0000x reference)
"""Trainium2 Bass kernel for nn_Encoding (vq_codebook / scaled-L2 softmax encoding).

Reference math (per batch b, with Xf = X[b] reshaped [D, N] and viewed [N, D]):
    sl[n,k] = s_k^2 * (||x_n||^2 - 2 <x_n, c_k> + ||c_k||^2)
    A = softmax_k(sl)
    E[k,d]  = sum_n A[n,k] * (x[n,d] - c[k,d])

Strategy (v2):
  - Data parallel over B: 4 batches per core x 8 cores.
  - Softmax shift: subtract the provable upper bound
    M[n] = (s2max + cmax)*x2[n] + cmax + vmax (linear in x2), giving
        sl'[n,k] = u'_k*x2[n] + xc'[n,k] + v'_k  <= 0
    with u' = s^2 - s2max - cmax, xc' = -2 s_k^2 <x,c_k>,
    v' = s^2 c2 - cmax - vmax.  Lower bound of the row max ~ -60, so exp
    neither overflows nor underflows; softmax is exact up to fp rounding.
  - HOST precomputes: X in bf16 in BOTH layouts ([d,n] for the xc matmul
    and [n,d] 128-row tiles with a baked ones-column for the aggregation),
    plus x2[n] = ||x_n||^2 in f32 EXACTLY (a partition-dim reduction the
    device engines cannot do cheaply), centered by XMEAN and split into
    bf16 hi/lo parts.  HBM traffic is ~2x X in bf16 = same bytes as one
    f32 copy of X; this is the memory-roofline currency of the problem.
  - Logits are assembled ENTIRELY in PSUM by the PE:
      * one rank-26 matmul per chunk adds u'_k*x2c[n] + v''_k using an
        error-compensated bf16 factorization
        (x2hi*uhi + x2hi*ulo + x2lo*uhi + 1*v''hi + 1*v''lo, < 1e-3 abs err),
        where x2c = x2 - XMEAN and v'' = v' + XMEAN*u'.
      * 8 xc matmuls per chunk (lhsT = bf16 X-tile, rhs = folded codewords).
    No DVE/ACT work at all for logits: ACT does one exp per chunk
    (PSUM -> bf16 H), DVE does one reduce (Z), one reciprocal (R), one
    bf16 cast and one broadcast multiply (A = H*R).
  - Aggregation: pE[k, 0:128] += A_j^T @ Xt_j ; pE[k, 128] += A_j^T @ 1
    via rhs = [Xt_j | ones-column] (the ones column is baked into the
    host-side Xt tiles), so sum_n A and sum_n A*x come from the same
    matmul.  E_final = pE[:, :D] - pE[:, D]*C on DVE per batch.

  Sync-wait budget: walrus fits only ONE sync wait per lowered
  instruction; the _legalize_waits pass hoists extras onto same-engine
  NOP/drain carriers (purely more conservative, no reordering).
"""

import sys

sys.path.insert(0, "/opt/trn_rl_repo")

import numpy as np
import ml_dtypes

import concourse.bass as bass
import concourse.tile as tile
from concourse import mybir
from concourse import bass_utils

D = 128
K = 32
B = 32
N = 9216  # 96*96
NCORES = 8
B_LOC = B // NCORES

CHUNK = 1024
NSUB = CHUNK // 128
NCHUNK = N // CHUNK
GRP = 3  # chunks per DMA load group
NROWS = 26  # x2hi*8, x2hi*8 (ulo), x2lo*8, ones (v''hi), ones (v''lo)
XMEAN = 128.0

F32 = mybir.dt.float32
BF16 = mybir.dt.bfloat16
FP8 = mybir.dt.float8e4
XB_SCALE = 8.0  # Xb = fp8(X / 8), cw = bf16(cw_folded * 8): keeps fp8 X in
# the well-conditioned range and halves the Xb HBM traffic vs bf16.


def _bcast_last(ap, n):
    """[P, F] -> [P, F, n] view with step-0 last dim."""
    return bass.AP(
        tensor=ap.tensor,
        offset=ap.offset,
        ap=[ap.ap[0], ap.ap[1], [0, n]],
    )


class _SplitDrainTC(tile.TileContext):
    """TileContext whose final drain splits its waits over several drain
    instructions: walrus only fits a couple of sync waits per instruction."""

    _WAITS_PER_DRAIN = 1

    def _drain_and_barrier(self, tick_clock, wait_clock):
        from concourse.vector_clock import ScopedClock, VectorClock
        from concourse.tile_sem_assignment import PROC_NAME_TO_IDX

        nproc = len(PROC_NAME_TO_IDX)
        gc = tick_clock.global_clock
        ticks = [gc[i] for i in range(nproc)]
        active = [i for i in range(nproc) if ticks[i] > 0]
        for group_start in range(0, len(active), self._WAITS_PER_DRAIN):
            group = active[group_start : group_start + self._WAITS_PER_DRAIN]
            partial = [0] * nproc
            for i in group:
                partial[i] = ticks[i]
            drain_inst = self.nc.sync.drain()
            wait_clock.add_sem_waits(
                drain_inst.ins, ScopedClock({None: VectorClock(partial)})
            )

        self.nc.all_engine_barrier()
        assert self.sems is not None
        popped = self.nc._tile_sem_poison_stack.pop()
        assert popped is self._sem_poison
        self.nc.clear_and_free_semaphores(list(self.sems.allocated().values()))
        self.nc.all_engine_barrier()


_ENGINE_ATTR = {
    "DVE": "vector",
    "Activation": "scalar",
    "PE": "tensor",
    "Pool": "gpsimd",
    "SP": "sync",
}


def _legalize_waits(nc):
    """Walrus codegen fits only ONE sync wait per lowered instruction.
    Hoist every extra wait onto an injected same-engine NOP/drain carrier
    placed directly before the over-budget instruction (purely more
    conservative: no reordering, identical semantics)."""
    from bass_rust import SyncInfo

    def make_carrier(engine_name):
        eng = getattr(nc, _ENGINE_ATTR[engine_name])
        bi = eng.engine_nop() if hasattr(eng, "engine_nop") else eng.drain()
        inst = bi.ins
        # Pull it back out of whatever block add_instruction appended to.
        for f in nc.m.functions:
            for b in f.blocks:
                il = b.instructions
                names = [x.name for x in il]
                if inst.name in names:
                    il2 = list(il)
                    il2.pop(names.index(inst.name))
                    b.instructions = il2
                    return inst
        raise AssertionError("carrier not found after append")

    n_carriers = 0
    for f in nc.m.functions:
        for b in f.blocks:
            il = list(b.instructions)
            out = []
            changed = False
            for inst in il:
                si = inst.sync_info
                waits = list(si.on_wait) if si is not None and si.on_wait else []
                if len(waits) > 1:
                    eng = str(inst.engine).split(".")[-1]
                    for w in waits[:-1]:
                        car = make_carrier(eng)
                        car.sync_info = SyncInfo(on_wait=[w], on_update=[])
                        out.append(car)
                        n_carriers += 1
                    inst.sync_info = SyncInfo(
                        on_wait=[waits[-1]],
                        on_update=list(si.on_update) if si.on_update else [],
                    )
                    changed = True
                out.append(inst)
            if changed:
                b.instructions = out
    return n_carriers


def build_nc(b_loc=B_LOC, n_cols=N):
    """Build the SPMD Bass program (same program on every core)."""
    nchunk = n_cols // CHUNK
    assert n_cols % CHUNK == 0

    nc = bass.Bass("TRN2", target_bir_lowering=False, debug=False)

    ngrp = nchunk // GRP
    assert nchunk % GRP == 0
    xb_dram = nc.dram_tensor(
        "Xb", [b_loc, ngrp, 128, GRP * CHUNK], FP8, kind="ExternalInput"
    ).ap()
    xt_dram = nc.dram_tensor(
        "Xt", [b_loc, ngrp, 128, GRP, NSUB, D + 1], BF16, kind="ExternalInput"
    ).ap()
    x2_dram = nc.dram_tensor(
        "x2p", [b_loc, ngrp, NROWS, GRP, 128], BF16, kind="ExternalInput"
    ).ap()
    uv_dram = nc.dram_tensor("uv", [NROWS, NSUB * K], BF16, kind="ExternalInput").ap()
    cw_dram = nc.dram_tensor("cw", [D, K], BF16, kind="ExternalInput").ap()
    cneg_dram = nc.dram_tensor("cneg", [K, D], F32, kind="ExternalInput").ap()
    e_dram = nc.dram_tensor("E", [b_loc, K, D], F32, kind="ExternalOutput").ap()

    with _SplitDrainTC(nc) as tc:
        with (
            tc.tile_pool(name="consts", bufs=1) as consts,
            tc.tile_pool(name="xin", bufs=5) as xin,
            tc.tile_pool(name="xtin", bufs=5) as xtin,
            tc.tile_pool(name="x2in", bufs=5) as x2in,
            tc.tile_pool(name="hp", bufs=4) as hp,
            tc.tile_pool(name="smalls", bufs=4) as smalls,
            tc.tile_pool(name="psum_sl", bufs=3, space="PSUM") as psum_sl,
            tc.tile_pool(name="psum_acc", bufs=2, space="PSUM") as psum_acc,
            tc.tile_pool(name="outp", bufs=4) as outp,
        ):
            uv = consts.tile([NROWS, NSUB * K], BF16)
            nc.sync.dma_start(out=uv, in_=uv_dram)
            cw = consts.tile([D, K], BF16)
            nc.sync.dma_start(out=cw, in_=cw_dram)
            cneg = consts.tile([K, D], F32)
            nc.sync.dma_start(out=cneg, in_=cneg_dram)
            # Startup dummy reads: pull the const-load DMA waits onto cheap
            # ops so steady-state compute never waits on a DMAHW semaphore.
            warm = consts.tile([1, 2], F32)
            nc.vector.tensor_copy(warm, cneg[0:1, 0:2])
            warm2 = consts.tile([1, 2], BF16)
            nc.vector.tensor_copy(warm2, uv[0:1, 0:2])
            warm3 = consts.tile([1, 2], BF16)
            nc.scalar.copy(warm3, cw[0:1, 0:2])

            for b in range(b_loc):
                pE = psum_acc.tile([K, D + 1], F32, tag="pE")

                for g in range(ngrp):
                    # Grouped loads: GRP chunks per DMA so descriptor
                    # generation (~128 descs / ~0.8us per load) amortizes
                    # over 3x the bytes.  All loads ride the Sync queue:
                    # it runs no compute, so no head-of-line blocking.
                    xg = xin.tile([128, GRP * CHUNK], FP8)
                    nc.sync.dma_start(out=xg, in_=xb_dram[b, g])
                    xtg = xtin.tile([128, GRP, NSUB, D + 1], BF16)
                    nc.sync.dma_start(out=xtg, in_=xt_dram[b, g])
                    x2g = x2in.tile([NROWS, GRP, 128], BF16)
                    nc.sync.dma_start(out=x2g, in_=x2_dram[b, g])

                    for cc in range(GRP):
                        c = g * GRP + cc
                        xf = xg[:, cc * CHUNK : (cc + 1) * CHUNK]
                        xt = xtg[:, cc]

                        # Logits in PSUM:  sl = (u'*x2c + v'') + xc'
                        sl = psum_sl.tile([128, NSUB, K], F32, tag="sl")
                        nc.tensor.matmul(
                            sl.rearrange("p j k -> p (j k)"),
                            lhsT=x2g[:, cc],
                            rhs=uv,
                            start=True,
                            stop=False,
                        )
                        for j in range(NSUB):
                            nc.tensor.matmul(
                                sl[:, j, :],
                                lhsT=xf[:, j * 128 : (j + 1) * 128],
                                rhs=cw,
                                start=False,
                                stop=(j == NSUB - 1),
                            )

                        # Softmax pieces: H = exp(sl) (bf16), R = 1/sum_k H,
                        # A = H * R (bf16).
                        H = hp.tile([128, NSUB, K], BF16, tag="H")
                        nc.scalar.activation(
                            H, sl, mybir.ActivationFunctionType.Exp
                        )
                        Z = smalls.tile([128, NSUB], F32, tag="Z")
                        nc.vector.reduce_sum(Z, H, axis=mybir.AxisListType.X)
                        R = smalls.tile([128, NSUB], F32, tag="R")
                        nc.vector.reciprocal(R, Z)
                        A = hp.tile([128, NSUB, K], BF16, tag="A")
                        nc.vector.tensor_tensor(
                            A, H, _bcast_last(R, K), mybir.AluOpType.mult
                        )

                        # pE[k, 0:128] += A_j^T @ Xt_j ; pE[k, 128] += A_j^T @ 1
                        for j in range(NSUB):
                            first = (c == 0) and (j == 0)
                            last = (c == nchunk - 1) and (j == NSUB - 1)
                            nc.tensor.matmul(
                                pE,
                                lhsT=A[:, j, :],
                                rhs=xt[:, j, :],
                                start=first,
                                stop=last,
                            )

                # E_final = pE[:, :D] - asum * C  ( = (cneg * asum) + pE )
                asum_sb = outp.tile([K, 1], F32, tag="asum")
                nc.vector.tensor_copy(asum_sb, pE[:, D : D + 1])
                e_sb = outp.tile([K, D], F32, tag="esb")
                nc.vector.scalar_tensor_tensor(
                    out=e_sb,
                    in0=cneg,
                    scalar=asum_sb,
                    in1=pE[:, 0:D],
                    op0=mybir.AluOpType.mult,
                    op1=mybir.AluOpType.add,
                )
                # SWDGE store keeps HWDGE queues free for the input streams.
                nc.gpsimd.dma_start(out=e_dram[b], in_=e_sb)

    n_car = _legalize_waits(nc)
    print(f"wait-legalizer inserted {n_car} carriers")
    return nc


def _split_bf16(x):
    """f32 -> (bf16 hi, bf16 lo) with hi+lo ~= x to ~2^-16 relative."""
    hi = x.astype(ml_dtypes.bfloat16)
    lo = (x - hi.astype(np.float32)).astype(ml_dtypes.bfloat16)
    return hi, lo


def _host_constants(codewords, scale):
    C = np.asarray(codewords, dtype=np.float32)
    s = np.asarray(scale, dtype=np.float32)
    s2 = s * s
    c2 = (C * C).sum(axis=1)
    cmax = float(np.sqrt(c2.max()))
    s2max = float(s2.max())
    v = s2 * c2
    vmax = float(v.max())
    u_p = s2 - (s2max + cmax)  # [K], <= 0
    v_pp = (v - (cmax + vmax)) + XMEAN * u_p  # [K]
    uhi, ulo = _split_bf16(u_p)
    vhi, vlo = _split_bf16(v_pp)

    uv = np.zeros((NROWS, NSUB, K), dtype=np.float32)
    for j in range(NSUB):
        uv[j, j, :] = uhi.astype(np.float32)
        uv[8 + j, j, :] = ulo.astype(np.float32)
        uv[16 + j, j, :] = uhi.astype(np.float32)
    uv[24, :, :] = vhi.astype(np.float32)[None, :]
    uv[25, :, :] = vlo.astype(np.float32)[None, :]

    cwf = (-2.0 * s2)[None, :] * C.T * XB_SCALE  # [D, K]
    return {
        "uv": uv.reshape(NROWS, NSUB * K).astype(ml_dtypes.bfloat16),
        "cw": cwf.astype(ml_dtypes.bfloat16),
        "cneg": (-C).astype(np.float32),
    }


def _host_x_tensors(X):
    """Build the per-batch device tensors from full f32 X [B, D, H, W].

    All three tensors are laid out so one GRP-chunk DMA load reads
    contiguous bytes per SBUF partition (full-rate descriptors)."""
    Xr = np.asarray(X, dtype=np.float32).reshape(B, D, N)
    ngrp = NCHUNK // GRP

    # exact f32 x2, centered, split hi/lo
    x2 = np.einsum("bdn,bdn->bn", Xr, Xr, dtype=np.float32)  # [B, N]
    x2c = x2 - XMEAN
    x2hi, x2lo = _split_bf16(x2c)
    # x2p [B, ngrp, NROWS, GRP, 128]; row j pairs with uhi, 8+j with ulo
    # (same x2hi data), 16+j with uhi (x2lo data), 24/25 are ones.
    x2hi_r = x2hi.reshape(B, ngrp, GRP, NSUB, 128).transpose(0, 1, 3, 2, 4)
    x2lo_r = x2lo.reshape(B, ngrp, GRP, NSUB, 128).transpose(0, 1, 3, 2, 4)
    x2p = np.empty((B, ngrp, NROWS, GRP, 128), dtype=ml_dtypes.bfloat16)
    x2p[:, :, 0:8] = x2hi_r
    x2p[:, :, 8:16] = x2hi_r
    x2p[:, :, 16:24] = x2lo_r
    x2p[:, :, 24:26] = np.asarray(1.0, dtype=ml_dtypes.bfloat16)

    Xbf = Xr.astype(ml_dtypes.bfloat16)  # [B, D, N] (for Xt)
    # Xb [B, ngrp, 128, GRP*CHUNK] fp8: per-partition contiguous group slab
    Xb = np.ascontiguousarray(
        (Xr / XB_SCALE)
        .astype(ml_dtypes.float8_e4m3fn)
        .reshape(B, D, ngrp, GRP * CHUNK)
        .transpose(0, 2, 1, 3)
    )

    # Xt tiles: xt[b, g, p, cc, j, d] = bf16(X[b, d, (g*GRP+cc)*1024 + j*128 + p]),
    # with a ones column at d = D.
    Xt_src = (
        Xbf.transpose(0, 2, 1)  # [B, N, D]
        .reshape(B, ngrp, GRP, NSUB, 128, D)
        .transpose(0, 1, 4, 2, 3, 5)  # [B, ngrp, 128, GRP, NSUB, D]
    )
    Xt = np.empty((B, ngrp, 128, GRP, NSUB, D + 1), dtype=ml_dtypes.bfloat16)
    Xt[..., :D] = Xt_src
    Xt[..., D] = np.asarray(1.0, dtype=ml_dtypes.bfloat16)
    return Xb, Xt, x2p


def build_in_maps(X, codewords, scale):
    consts = _host_constants(codewords, scale)
    Xb, Xt, x2p = _host_x_tensors(X)
    in_maps = []
    for i in range(NCORES):
        m = dict(consts)
        sl = slice(i * B_LOC, (i + 1) * B_LOC)
        m["Xb"] = np.ascontiguousarray(Xb[sl])
        m["Xt"] = np.ascontiguousarray(Xt[sl])
        m["x2p"] = np.ascontiguousarray(x2p[sl])
        in_maps.append(m)
    return in_maps


_NC_CACHE = {}


def _get_nc():
    key = (B_LOC, N)
    if key not in _NC_CACHE:
        _NC_CACHE[key] = build_nc(*key)
    return _NC_CACHE[key]


def kernel(X, codewords, scale):
    in_maps = build_in_maps(X, codewords, scale)
    nc = _get_nc()
    res = bass_utils.run_bass_kernel_spmd(nc, in_maps, list(range(NCORES)))
    E = np.concatenate([res.results[i]["E"] for i in range(NCORES)], axis=0)
    return E.astype(np.float32)


if __name__ == "__main__":
    rng = np.random.default_rng(0)
    X = rng.standard_normal((B, D, 96, 96), dtype=np.float32)
    cwds = rng.uniform(-1 / 64, 1 / 64, size=(K, D)).astype(np.float32)
    sc = rng.uniform(-1.0, 0.0, size=(K,)).astype(np.float32)
    E = kernel(X=X, codewords=cwds, scale=sc)
    print("E", E.shape, E.dtype, np.abs(E).mean())


# revision 17
# speedup vs baseline: 1.3753x; 1.3753x over previous
"""Trainium2 Bass kernel for nn_Encoding (vq_codebook / scaled-L2 softmax encoding).

Reference math (per batch b, with Xf = X[b] reshaped [D, N] and viewed [N, D]):
    sl[n,k] = s_k^2 * (||x_n||^2 - 2 <x_n, c_k> + ||c_k||^2)
    A = softmax_k(sl)
    E[k,d]  = sum_n A[n,k] * (x[n,d] - c[k,d])

Strategy (v2):
  - Data parallel over B: 4 batches per core x 8 cores.
  - Softmax shift: subtract the provable upper bound
    M[n] = (s2max + cmax)*x2[n] + cmax + vmax (linear in x2), giving
        sl'[n,k] = u'_k*x2[n] + xc'[n,k] + v'_k  <= 0
    with u' = s^2 - s2max - cmax, xc' = -2 s_k^2 <x,c_k>,
    v' = s^2 c2 - cmax - vmax.  Lower bound of the row max ~ -60, so exp
    neither overflows nor underflows; softmax is exact up to fp rounding.
  - HOST precomputes: X in bf16 in BOTH layouts ([d,n] for the xc matmul
    and [n,d] 128-row tiles with a baked ones-column for the aggregation),
    plus x2[n] = ||x_n||^2 in f32 EXACTLY (a partition-dim reduction the
    device engines cannot do cheaply), centered by XMEAN and split into
    bf16 hi/lo parts.  HBM traffic is ~2x X in bf16 = same bytes as one
    f32 copy of X; this is the memory-roofline currency of the problem.
  - Logits are assembled ENTIRELY in PSUM by the PE:
      * one rank-26 matmul per chunk adds u'_k*x2c[n] + v''_k using an
        error-compensated bf16 factorization
        (x2hi*uhi + x2hi*ulo + x2lo*uhi + 1*v''hi + 1*v''lo, < 1e-3 abs err),
        where x2c = x2 - XMEAN and v'' = v' + XMEAN*u'.
      * 8 xc matmuls per chunk (lhsT = bf16 X-tile, rhs = folded codewords).
    No DVE/ACT work at all for logits: ACT does one exp per chunk
    (PSUM -> bf16 H), DVE does one reduce (Z), one reciprocal (R), one
    bf16 cast and one broadcast multiply (A = H*R).
  - Aggregation: pE[k, 0:128] += A_j^T @ Xt_j ; pE[k, 128] += A_j^T @ 1
    via rhs = [Xt_j | ones-column] (the ones column is baked into the
    host-side Xt tiles), so sum_n A and sum_n A*x come from the same
    matmul.  E_final = pE[:, :D] - pE[:, D]*C on DVE per batch.

  Sync-wait budget: walrus fits only ONE sync wait per lowered
  instruction; the _legalize_waits pass hoists extras onto same-engine
  NOP/drain carriers (purely more conservative, no reordering).
"""

import sys

sys.path.insert(0, "/opt/trn_rl_repo")

import numpy as np
import ml_dtypes

import concourse.bass as bass
import concourse.tile as tile
from concourse import mybir
from concourse import bass_utils

D = 128
K = 32
B = 32
N = 9216  # 96*96
NCORES = 8
B_LOC = B // NCORES

CHUNK = 1024
NSUB = CHUNK // 128
NCHUNK = N // CHUNK
GRP = 3  # chunks per DMA load group
NROWS = 26  # x2hi*8, x2hi*8 (ulo), x2lo*8, ones (v''hi), ones (v''lo)
XMEAN = 128.0

F32 = mybir.dt.float32
BF16 = mybir.dt.bfloat16
FP8 = mybir.dt.float8e4
XB_SCALE = 8.0  # Xb = fp8(X / 8), cw = bf16(cw_folded * 8): keeps fp8 X in
# the well-conditioned range and halves the Xb HBM traffic vs bf16.


def _bcast_last(ap, n):
    """[P, F] -> [P, F, n] view with step-0 last dim."""
    return bass.AP(
        tensor=ap.tensor,
        offset=ap.offset,
        ap=[ap.ap[0], ap.ap[1], [0, n]],
    )


class _SplitDrainTC(tile.TileContext):
    """TileContext whose final drain splits its waits over several drain
    instructions: walrus only fits a couple of sync waits per instruction."""

    _WAITS_PER_DRAIN = 1

    def _drain_and_barrier(self, tick_clock, wait_clock):
        from concourse.vector_clock import ScopedClock, VectorClock
        from concourse.tile_sem_assignment import PROC_NAME_TO_IDX

        nproc = len(PROC_NAME_TO_IDX)
        gc = tick_clock.global_clock
        ticks = [gc[i] for i in range(nproc)]
        active = [i for i in range(nproc) if ticks[i] > 0]
        for group_start in range(0, len(active), self._WAITS_PER_DRAIN):
            group = active[group_start : group_start + self._WAITS_PER_DRAIN]
            partial = [0] * nproc
            for i in group:
                partial[i] = ticks[i]
            drain_inst = self.nc.sync.drain()
            wait_clock.add_sem_waits(
                drain_inst.ins, ScopedClock({None: VectorClock(partial)})
            )

        self.nc.all_engine_barrier()
        assert self.sems is not None
        popped = self.nc._tile_sem_poison_stack.pop()
        assert popped is self._sem_poison
        self.nc.clear_and_free_semaphores(list(self.sems.allocated().values()))
        self.nc.all_engine_barrier()


_ENGINE_ATTR = {
    "DVE": "vector",
    "Activation": "scalar",
    "PE": "tensor",
    "Pool": "gpsimd",
    "SP": "sync",
}


def _legalize_waits(nc):
    """Walrus codegen fits only ONE sync wait per lowered instruction.
    Hoist every extra wait onto an injected same-engine NOP/drain carrier
    placed directly before the over-budget instruction (purely more
    conservative: no reordering, identical semantics)."""
    from bass_rust import SyncInfo

    def make_carrier(engine_name):
        eng = getattr(nc, _ENGINE_ATTR[engine_name])
        bi = eng.engine_nop() if hasattr(eng, "engine_nop") else eng.drain()
        inst = bi.ins
        # Pull it back out of whatever block add_instruction appended to.
        for f in nc.m.functions:
            for b in f.blocks:
                il = b.instructions
                names = [x.name for x in il]
                if inst.name in names:
                    il2 = list(il)
                    il2.pop(names.index(inst.name))
                    b.instructions = il2
                    return inst
        raise AssertionError("carrier not found after append")

    n_carriers = 0
    for f in nc.m.functions:
        for b in f.blocks:
            il = list(b.instructions)
            out = []
            changed = False
            for inst in il:
                si = inst.sync_info
                waits = list(si.on_wait) if si is not None and si.on_wait else []
                if len(waits) > 1:
                    eng = str(inst.engine).split(".")[-1]
                    for w in waits[:-1]:
                        car = make_carrier(eng)
                        car.sync_info = SyncInfo(on_wait=[w], on_update=[])
                        out.append(car)
                        n_carriers += 1
                    inst.sync_info = SyncInfo(
                        on_wait=[waits[-1]],
                        on_update=list(si.on_update) if si.on_update else [],
                    )
                    changed = True
                out.append(inst)
            if changed:
                b.instructions = out
    return n_carriers


def build_nc(b_loc=B_LOC, n_cols=N):
    """Build the SPMD Bass program (same program on every core)."""
    nchunk = n_cols // CHUNK
    assert n_cols % CHUNK == 0

    nc = bass.Bass("TRN2", target_bir_lowering=False, debug=False)

    ngrp = nchunk // GRP
    assert nchunk % GRP == 0
    xb_dram = nc.dram_tensor(
        "Xb", [b_loc, ngrp, 128, GRP * CHUNK], FP8, kind="ExternalInput"
    ).ap()
    xt_dram = nc.dram_tensor(
        "Xt", [b_loc, ngrp, 128, GRP, NSUB, D + 1], BF16, kind="ExternalInput"
    ).ap()
    x2_dram = nc.dram_tensor(
        "x2p", [b_loc, ngrp, NROWS, GRP, 128], BF16, kind="ExternalInput"
    ).ap()
    uv_dram = nc.dram_tensor("uv", [NROWS, NSUB * K], BF16, kind="ExternalInput").ap()
    cw_dram = nc.dram_tensor("cw", [D, K], BF16, kind="ExternalInput").ap()
    cneg_dram = nc.dram_tensor("cneg", [K, D], F32, kind="ExternalInput").ap()
    e_dram = nc.dram_tensor("E", [b_loc, K, D], F32, kind="ExternalOutput").ap()

    with _SplitDrainTC(nc) as tc:
        with (
            tc.tile_pool(name="consts", bufs=1) as consts,
            tc.tile_pool(name="xin", bufs=5) as xin,
            tc.tile_pool(name="xtin", bufs=5) as xtin,
            tc.tile_pool(name="x2in", bufs=5) as x2in,
            tc.tile_pool(name="hp", bufs=4) as hp,
            tc.tile_pool(name="smalls", bufs=4) as smalls,
            tc.tile_pool(name="psum_sl", bufs=3, space="PSUM") as psum_sl,
            tc.tile_pool(name="psum_acc", bufs=2, space="PSUM") as psum_acc,
            tc.tile_pool(name="outp", bufs=4) as outp,
        ):
            uv = consts.tile([NROWS, NSUB * K], BF16)
            nc.sync.dma_start(out=uv, in_=uv_dram)
            cw = consts.tile([D, K], BF16)
            nc.sync.dma_start(out=cw, in_=cw_dram)
            cneg = consts.tile([K, D], F32)
            nc.sync.dma_start(out=cneg, in_=cneg_dram)
            # Startup dummy reads: pull the const-load DMA waits onto cheap
            # ops so steady-state compute never waits on a DMAHW semaphore.
            warm = consts.tile([1, 2], F32)
            nc.vector.tensor_copy(warm, cneg[0:1, 0:2])
            warm2 = consts.tile([1, 2], BF16)
            nc.vector.tensor_copy(warm2, uv[0:1, 0:2])
            warm3 = consts.tile([1, 2], BF16)
            nc.scalar.copy(warm3, cw[0:1, 0:2])

            for b in range(b_loc):
                pE = psum_acc.tile([K, D + 1], F32, tag="pE")

                for g in range(ngrp):
                    # Grouped loads: GRP chunks per DMA so descriptor
                    # generation (~128 descs / ~0.8us per load) amortizes
                    # over 3x the bytes.  Xt rides the Scalar queue so the
                    # two big streams issue and stall independently.
                    xg = xin.tile([128, GRP * CHUNK], FP8)
                    nc.sync.dma_start(out=xg, in_=xb_dram[b, g])
                    xtg = xtin.tile([128, GRP, NSUB, D + 1], BF16)
                    nc.scalar.dma_start(out=xtg, in_=xt_dram[b, g])
                    x2g = x2in.tile([NROWS, GRP, 128], BF16)
                    nc.sync.dma_start(out=x2g, in_=x2_dram[b, g])

                    for cc in range(GRP):
                        c = g * GRP + cc
                        xf = xg[:, cc * CHUNK : (cc + 1) * CHUNK]
                        xt = xtg[:, cc]

                        # Logits in PSUM:  sl = (u'*x2c + v'') + xc'
                        sl = psum_sl.tile([128, NSUB, K], F32, tag="sl")
                        nc.tensor.matmul(
                            sl.rearrange("p j k -> p (j k)"),
                            lhsT=x2g[:, cc],
                            rhs=uv,
                            start=True,
                            stop=False,
                        )
                        for j in range(NSUB):
                            nc.tensor.matmul(
                                sl[:, j, :],
                                lhsT=xf[:, j * 128 : (j + 1) * 128],
                                rhs=cw,
                                start=False,
                                stop=(j == NSUB - 1),
                            )

                        # Softmax pieces: H = exp(sl) (bf16), R = 1/sum_k H,
                        # A = H * R (bf16).
                        H = hp.tile([128, NSUB, K], BF16, tag="H")
                        nc.scalar.activation(
                            H, sl, mybir.ActivationFunctionType.Exp
                        )
                        Z = smalls.tile([128, NSUB], F32, tag="Z")
                        nc.vector.reduce_sum(Z, H, axis=mybir.AxisListType.X)
                        R = smalls.tile([128, NSUB], F32, tag="R")
                        nc.vector.reciprocal(R, Z)
                        A = hp.tile([128, NSUB, K], BF16, tag="A")
                        nc.vector.tensor_tensor(
                            A, H, _bcast_last(R, K), mybir.AluOpType.mult
                        )

                        # pE[k, 0:128] += A_j^T @ Xt_j ; pE[k, 128] += A_j^T @ 1
                        for j in range(NSUB):
                            first = (c == 0) and (j == 0)
                            last = (c == nchunk - 1) and (j == NSUB - 1)
                            nc.tensor.matmul(
                                pE,
                                lhsT=A[:, j, :],
                                rhs=xt[:, j, :],
                                start=first,
                                stop=last,
                            )

                # E_final = pE[:, :D] - asum * C  ( = (cneg * asum) + pE )
                asum_sb = outp.tile([K, 1], F32, tag="asum")
                nc.vector.tensor_copy(asum_sb, pE[:, D : D + 1])
                e_sb = outp.tile([K, D], F32, tag="esb")
                nc.vector.scalar_tensor_tensor(
                    out=e_sb,
                    in0=cneg,
                    scalar=asum_sb,
                    in1=pE[:, 0:D],
                    op0=mybir.AluOpType.mult,
                    op1=mybir.AluOpType.add,
                )
                # SWDGE store keeps HWDGE queues free for the input streams.
                nc.gpsimd.dma_start(out=e_dram[b], in_=e_sb)

    n_car = _legalize_waits(nc)
    print(f"wait-legalizer inserted {n_car} carriers")
    return nc


def _split_bf16(x):
    """f32 -> (bf16 hi, bf16 lo) with hi+lo ~= x to ~2^-16 relative."""
    hi = x.astype(ml_dtypes.bfloat16)
    lo = (x - hi.astype(np.float32)).astype(ml_dtypes.bfloat16)
    return hi, lo


def _host_constants(codewords, scale):
    C = np.asarray(codewords, dtype=np.float32)
    s = np.asarray(scale, dtype=np.float32)
    s2 = s * s
    c2 = (C * C).sum(axis=1)
    cmax = float(np.sqrt(c2.max()))
    s2max = float(s2.max())
    v = s2 * c2
    vmax = float(v.max())
    u_p = s2 - (s2max + cmax)  # [K], <= 0
    v_pp = (v - (cmax + vmax)) + XMEAN * u_p  # [K]
    uhi, ulo = _split_bf16(u_p)
    vhi, vlo = _split_bf16(v_pp)

    uv = np.zeros((NROWS, NSUB, K), dtype=np.float32)
    for j in range(NSUB):
        uv[j, j, :] = uhi.astype(np.float32)
        uv[8 + j, j, :] = ulo.astype(np.float32)
        uv[16 + j, j, :] = uhi.astype(np.float32)
    uv[24, :, :] = vhi.astype(np.float32)[None, :]
    uv[25, :, :] = vlo.astype(np.float32)[None, :]

    cwf = (-2.0 * s2)[None, :] * C.T * XB_SCALE  # [D, K]
    return {
        "uv": uv.reshape(NROWS, NSUB * K).astype(ml_dtypes.bfloat16),
        "cw": cwf.astype(ml_dtypes.bfloat16),
        "cneg": (-C).astype(np.float32),
    }


def _host_x_tensors(X):
    """Build the per-batch device tensors from full f32 X [B, D, H, W].

    All three tensors are laid out so one GRP-chunk DMA load reads
    contiguous bytes per SBUF partition (full-rate descriptors)."""
    Xr = np.asarray(X, dtype=np.float32).reshape(B, D, N)
    ngrp = NCHUNK // GRP

    # exact f32 x2, centered, split hi/lo
    x2 = np.einsum("bdn,bdn->bn", Xr, Xr, dtype=np.float32)  # [B, N]
    x2c = x2 - XMEAN
    x2hi, x2lo = _split_bf16(x2c)
    # x2p [B, ngrp, NROWS, GRP, 128]; row j pairs with uhi, 8+j with ulo
    # (same x2hi data), 16+j with uhi (x2lo data), 24/25 are ones.
    x2hi_r = x2hi.reshape(B, ngrp, GRP, NSUB, 128).transpose(0, 1, 3, 2, 4)
    x2lo_r = x2lo.reshape(B, ngrp, GRP, NSUB, 128).transpose(0, 1, 3, 2, 4)
    x2p = np.empty((B, ngrp, NROWS, GRP, 128), dtype=ml_dtypes.bfloat16)
    x2p[:, :, 0:8] = x2hi_r
    x2p[:, :, 8:16] = x2hi_r
    x2p[:, :, 16:24] = x2lo_r
    x2p[:, :, 24:26] = np.asarray(1.0, dtype=ml_dtypes.bfloat16)

    Xbf = Xr.astype(ml_dtypes.bfloat16)  # [B, D, N] (for Xt)
    # Xb [B, ngrp, 128, GRP*CHUNK] fp8: per-partition contiguous group slab
    Xb = np.ascontiguousarray(
        (Xr / XB_SCALE)
        .astype(ml_dtypes.float8_e4m3fn)
        .reshape(B, D, ngrp, GRP * CHUNK)
        .transpose(0, 2, 1, 3)
    )

    # Xt tiles: xt[b, g, p, cc, j, d] = bf16(X[b, d, (g*GRP+cc)*1024 + j*128 + p]),
    # with a ones column at d = D.
    Xt_src = (
        Xbf.transpose(0, 2, 1)  # [B, N, D]
        .reshape(B, ngrp, GRP, NSUB, 128, D)
        .transpose(0, 1, 4, 2, 3, 5)  # [B, ngrp, 128, GRP, NSUB, D]
    )
    Xt = np.empty((B, ngrp, 128, GRP, NSUB, D + 1), dtype=ml_dtypes.bfloat16)
    Xt[..., :D] = Xt_src
    Xt[..., D] = np.asarray(1.0, dtype=ml_dtypes.bfloat16)
    return Xb, Xt, x2p


def build_in_maps(X, codewords, scale):
    consts = _host_constants(codewords, scale)
    Xb, Xt, x2p = _host_x_tensors(X)
    in_maps = []
    for i in range(NCORES):
        m = dict(consts)
        sl = slice(i * B_LOC, (i + 1) * B_LOC)
        m["Xb"] = np.ascontiguousarray(Xb[sl])
        m["Xt"] = np.ascontiguousarray(Xt[sl])
        m["x2p"] = np.ascontiguousarray(x2p[sl])
        in_maps.append(m)
    return in_maps


_NC_CACHE = {}


def _get_nc():
    key = (B_LOC, N)
    if key not in _NC_CACHE:
        _NC_CACHE[key] = build_nc(*key)
    return _NC_CACHE[key]


def kernel(X, codewords, scale):
    in_maps = build_in_maps(X, codewords, scale)
    nc = _get_nc()
    res = bass_utils.run_bass_kernel_spmd(nc, in_maps, list(range(NCORES)))
    E = np.concatenate([res.results[i]["E"] for i in range(NCORES)], axis=0)
    return E.astype(np.float32)


if __name__ == "__main__":
    rng = np.random.default_rng(0)
    X = rng.standard_normal((B, D, 96, 96), dtype=np.float32)
    cwds = rng.uniform(-1 / 64, 1 / 64, size=(K, D)).astype(np.float32)
    sc = rng.uniform(-1.0, 0.0, size=(K,)).astype(np.float32)
    E = kernel(X=X, codewords=cwds, scale=sc)
    print("E", E.shape, E.dtype, np.abs(E).mean())
